# revision 3
# baseline (speedup 1.0000x reference)
"""BlockDecay (RetNet-style chunkwise linear attention with per-feature decay)
Trainium2 Bass kernel, batch-parallel over 8 NeuronCores.

Math (per batch): out[t] = sum_r q[t,r] * S_t[r,:],
  S_t[r,d] = sum_{s<=t} gamma_r^{t-s} k[s,r] h[s,d]
computed chunkwise with C=128 using the standard factorization
  A[i,j] = (q gamma^i) . (k gamma^-j),  intra = (A*mask) @ h,
  inter  = (q gamma^i) @ S,   S' = gamma^C S + K',
  K'[r,d] = sum_j gamma_r^{C-j} k[j,r] h[j,d]   (S carries a folded
  gamma^1 so inter needs no extra scale)

v13: all-bf16 PE stream (1 cyc/row + FWL; PSUM accumulates fp32) with
the PSUM->SBUF crossings batched to amortize the fixed DVE/ACT op
costs (TRN2 errata: SBUF-src ops ~2.3x spec; PSUM-src DVE op =
(120+FD)/0.96GHz):
  - A blocks: 8 chunks into one [128,1024] PSUM tile, ONE DVE
    mask-multiply per 8 chunks (tri8 is the causal mask replicated).
  - K' blocks: 4 chunks into one [128,512] PSUM tile, ONE ACT copy
    to SBUF bf16; the serial state recurrence then runs on DVE as
    all-SBUF bf16 scalar_tensor_tensor (2x perf mode).
  - OT blocks: 4 chunks accumulated in one [128,512] PSUM tile, ONE
    ACT copy to the bf16 output staging buffer.
Group sizes taper near the end (4/2/1/1) to shorten the tail chain
behind the last DMA piece.  Output DMA pieces ride the sync queue.

Host pre-scales/transposes all operands; device layout:
  qsT [R, W] bf16 = (q * gamma^(i%C)).T
  ksT [R, W] bf16 = (k * gamma^-(j%C)).T
  k2n [128, W] bf16  block-local [j, (blk, r)] = k*gamma^(C - j%C)
  hn  [128, W] bf16  block-local [j, (blk, d)]
  tri [128, 128] f32 causal mask transposed (tri[j,i] = i>=j)
  g128 [128, 1] f32 = gamma^C
Output otT [D, W] bf16 (transposed), host transposes + upcasts.
Measured rel err (absmax norm) ~4e-3 vs the 2e-2 gate.
"""
import os
import sys
import numpy as np

for _p in ("/root/.axon_site", "/root/.axon_site/_ro/trn_rl_repo",
           "/root/.axon_site/_ro/pypackages"):
    if _p not in sys.path and os.path.isdir(_p):
        sys.path.append(_p)

B, W, R, D = 8, 4096, 128, 128
C = 128
NBLK = W // C

# chunk groups: [a, b) ranges
G_MSK = [(0, 8), (8, 16), (16, 24), (24, 28), (28, 30), (30, 31), (31, 32)]
G_CP = [(0, 4), (4, 8), (8, 12), (12, 16), (16, 20), (20, 24), (24, 28),
        (28, 30), (30, 31), (31, 32)]
# output DMA piece [lo, hi) in chunks, issued after the OT copy of the
# cp-group ending at chunk hi-1
_OUT_PIECES = {8: (0, 8), 16: (8, 16), 24: (16, 24), 28: (24, 28),
               30: (28, 30), 31: (30, 31), 32: (31, 32)}

_PROG = {}


def _patched_tc(nc):
    """TileContext with a cheap exit: per-sem single-wait drains on sync
    (this walrus accepts one sync-wait per instruction, and a blocking
    drain on an early-finishing engine stalls SWDGE descriptor handling),
    one barrier, then sem clears for idempotent re-execution.  The final
    join is walrus's own BSP model-end sync."""
    import concourse.tile as tile
    import concourse.tile_sem_assignment as tsa
    from concourse.tile import ScopedClock

    class PatchedTileContext(tile.TileContext):
        def _drain_and_barrier(self, tick_clock, wait_clock):
            gc = tick_clock.global_clock
            n = tsa.N_PROCS
            nc = self.nc
            for p in range(n):
                ticks = gc[p]
                if ticks <= 0:
                    continue
                d = nc.sync.drain()
                wait_clock.add_sem_waits(
                    d.ins,
                    ScopedClock({None: tsa.VectorClock(
                        [ticks if q == p else 0 for q in range(n)])}),
                )
            nc.all_engine_barrier()
            assert self.sems is not None
            popped = nc._tile_sem_poison_stack.pop()
            assert popped is self._sem_poison
            nc.clear_and_free_semaphores(list(self.sems.allocated().values()))

    return PatchedTileContext(nc)


def _split_multi_waits(nc, limit=1):
    """Hoist extra sync-waits onto injected same-engine NoOps (in-order
    engines make waiting earlier in the stream safe)."""
    import concourse.mybir as mybir
    n_new = 0
    for fn in nc.m.functions:
        for bb in fn.blocks:
            out = []
            changed = False
            for inst in bb.instructions:
                si = getattr(inst, "sync_info", None)
                waits = list(si.on_wait) if si is not None and si.on_wait else []
                if len(waits) > limit:
                    for w in waits[:-limit]:
                        nop = mybir.InstNoOp(
                            name=f"I-wsplit-{n_new}",
                            engine=inst.engine,
                            sync_info=mybir.SyncInfo(on_wait=[w], on_update=[]),
                        )
                        n_new += 1
                        out.append(nop)
                    si.on_wait = waits[-limit:]
                    changed = True
                out.append(inst)
            if changed:
                bb.instructions = out
    return n_new


def _build_program():
    key = "v13"
    if key in _PROG:
        return _PROG[key]
    import concourse.bass as bass
    import concourse.mybir as mybir

    F32 = mybir.dt.float32
    BF = mybir.dt.bfloat16
    n_warm = int(os.environ.get("BD_NWARM", "12"))

    nc = bass.Bass()
    qsT = nc.declare_dram_parameter("qsT", [128, W], BF, isOutput=False)
    ksT = nc.declare_dram_parameter("ksT", [128, W], BF, isOutput=False)
    k2n = nc.declare_dram_parameter("k2n", [128, W], BF, isOutput=False)
    hn = nc.declare_dram_parameter("hn", [128, W], BF, isOutput=False)
    tri = nc.declare_dram_parameter("tri", [128, 128], F32, isOutput=False)
    g128 = nc.declare_dram_parameter("g128", [128, 1], F32, isOutput=False)
    otT = nc.declare_dram_parameter("otT", [128, W], BF, isOutput=True)

    mm = nc.tensor.matmul
    with _patched_tc(nc) as tc:
        with tc.tile_pool(name="big", bufs=1) as big, \
             tc.tile_pool(name="small", bufs=1) as small, \
             tc.tile_pool(name="st", bufs=12) as stp, \
             tc.tile_pool(name="amp", bufs=3) as amp, \
             tc.tile_pool(name="kpsb", bufs=2) as kpsb, \
             tc.tile_pool(name="ps_at", bufs=2, space="PSUM") as ps_at, \
             tc.tile_pool(name="ps_ot", bufs=2, space="PSUM") as ps_ot, \
             tc.tile_pool(name="ps_kp", bufs=2, space="PSUM") as ps_kp:

            qsT_sb = big.tile([128, W], BF, tag="qsT")
            ksT_sb = big.tile([128, W], BF, tag="ksT")
            k2n_sb = big.tile([128, W], BF, tag="k2n")
            hn_sb = big.tile([128, W], BF, tag="hn")
            otT_sb = big.tile([128, W], BF, tag="otT")
            tri_sb = small.tile([128, 128], F32, tag="tri")
            tri8_sb = small.tile([128, 1024], BF, tag="tri8")
            g128_sb = small.tile([128, 1], F32, tag="g128")

            # PE warm-up: dummy matmuls fill the DMA-wait window and flip
            # the HAM clock gate to 8/8 before the real stream starts.
            wz = small.tile([128, 512], BF, tag="wz")
            nc.vector.memset(wz[:], 0.0)
            for _ in range(n_warm):
                wp = ps_ot.tile([128, 512], F32, tag="ot")
                mm(wp[:], wz[:, :128], wz[:], start=True, stop=True)

            # inputs split across the HWDGE ring (sync) and the SWDGE ring
            # (gpsimd); consts ride on scalar; output pieces on sync
            # (issued after its input issues are done).
            nc.scalar.dma_start(tri_sb[:], tri[:])
            nc.scalar.dma_start(g128_sb[:], g128[:])
            P = W // 4
            for p in range(4):
                s = slice(p * P, (p + 1) * P)
                nc.sync.dma_start(k2n_sb[:, s], k2n[:, s])
                nc.gpsimd.dma_start(hn_sb[:, s], hn[:, s])
                nc.sync.dma_start(ksT_sb[:, s], ksT[:, s])
                nc.gpsimd.dma_start(qsT_sb[:, s], qsT[:, s])

            # replicate the causal mask x8 (bf16) while DMA streams in
            for i in range(8):
                nc.scalar.copy(tri8_sb[:, i * 128:(i + 1) * 128], tri_sb[:])

            S_prev = stp.tile([128, 128], BF, tag="S")
            nc.vector.memset(S_prev[:], 0.0)
            S_at = {0: S_prev}

            cp_iter = iter(G_CP)
            cp_done = 0
            for (a, b) in G_MSK:
                # K' matmuls + batched PSUM->SBUF copy + state recurrence
                while cp_done < b:
                    ca, cb = next(cp_iter)
                    n = cb - ca
                    kp_ps = ps_kp.tile([128, 512], F32, tag="kp")
                    for m in range(ca, cb):
                        r = m - ca
                        jj = slice(m * 128, (m + 1) * 128)
                        mm(kp_ps[:, r * 128:(r + 1) * 128], k2n_sb[:, jj],
                           hn_sb[:, jj], start=True, stop=True)
                    kp_sb = kpsb.tile([128, 512], BF, tag="kpsb")
                    nc.scalar.copy(kp_sb[:, :n * 128], kp_ps[:, :n * 128])
                    for m in range(ca, cb):
                        r = m - ca
                        S_new = stp.tile([128, 128], BF, tag="S")
                        nc.vector.scalar_tensor_tensor(
                            out=S_new[:], in0=S_at[m][:],
                            scalar=g128_sb[:, 0:1],
                            in1=kp_sb[:, r * 128:(r + 1) * 128],
                            op0=mybir.AluOpType.mult,
                            op1=mybir.AluOpType.add)
                        S_at[m + 1] = S_new
                    cp_done = cb

                # A matmuls for the mask group, one batched mask-multiply
                n = b - a
                atb = ps_at.tile([128, 1024], F32, tag="at")
                for m in range(a, b):
                    r = m - a
                    jj = slice(m * 128, (m + 1) * 128)
                    mm(atb[:, r * 128:(r + 1) * 128], ksT_sb[:, jj],
                       qsT_sb[:, jj], start=True, stop=True)
                am = amp.tile([128, 1024], BF, tag="am")
                nc.vector.tensor_mul(am[:, :n * 128], atb[:, :n * 128],
                                     tri8_sb[:, :n * 128])

                # intra + inter into OT PSUM, copied out per cp-group
                for (ca, cb) in [g for g in G_CP if a <= g[0] and g[1] <= b]:
                    cn = cb - ca
                    ot_ps = ps_ot.tile([128, 512], F32, tag="ot")
                    for m in range(ca, cb):
                        r = m - ca
                        jj = slice(m * 128, (m + 1) * 128)
                        osl = slice(r * 128, (r + 1) * 128)
                        asl = slice((m - a) * 128, (m - a + 1) * 128)
                        mm(ot_ps[:, osl], hn_sb[:, jj], am[:, asl],
                           start=True, stop=False)
                        mm(ot_ps[:, osl], S_at[m][:], qsT_sb[:, jj],
                           start=False, stop=True)
                    oj = slice(ca * 128, cb * 128)
                    nc.scalar.copy(otT_sb[:, oj], ot_ps[:, :cn * 128])
                    if cb in _OUT_PIECES:
                        lo, hi = _OUT_PIECES[cb]
                        s = slice(lo * 128, hi * 128)
                        nc.sync.dma_start(otT[:, s], otT_sb[:, s])

    _split_multi_waits(nc)
    _PROG[key] = nc
    return nc


def _host_prep(q_alpha, k, h_norm, gamma_vec, causal_mask):
    import ml_dtypes
    bf = ml_dtypes.bfloat16
    gamma = np.clip(np.asarray(gamma_vec, np.float64), 1e-8, None)
    log_g = np.log(gamma)
    i_loc = (np.arange(W) % C).astype(np.float64)
    Sq = np.exp(np.outer(i_loc, log_g))          # [W, R] gamma^(i%C)
    Skneg = np.exp(np.outer(-i_loc, log_g))      # gamma^-(j%C)
    Sk2 = np.exp(np.outer(C - i_loc, log_g))     # gamma^(C - j%C)
    g128 = np.exp(C * log_g).astype(np.float32).reshape(128, 1)

    tri = np.ascontiguousarray(np.asarray(causal_mask, np.float32).T)

    def blockify(x):  # [W, 128] -> [128, (blk, 128)]
        return np.ascontiguousarray(
            x.reshape(NBLK, 128, 128).transpose(1, 0, 2).reshape(128, W))

    in_maps = []
    for b in range(B):
        q64 = np.asarray(q_alpha[b], np.float64)
        k64 = np.asarray(k[b], np.float64)
        in_maps.append({
            "qsT": np.ascontiguousarray((q64 * Sq).T.astype(bf)),
            "ksT": np.ascontiguousarray((k64 * Skneg).T.astype(bf)),
            "k2n": blockify((k64 * Sk2).astype(bf)),
            "hn": blockify(np.asarray(h_norm[b], bf)),
            "tri": tri,
            "g128": g128,
        })
    return in_maps


def _ensure_ntff_hook():
    try:
        from antenv import axon_hooks  # noqa: F401
        return
    except ImportError:
        pass
    import types
    import antenv
    try:
        import trn_agent_boot.trn_boot as tb
        hook = tb._ntff_profile_via_ctypes("/opt/axon/libaxon_pjrt.so")
    except Exception:
        hook = None
    mod = types.ModuleType("antenv.axon_hooks")
    mod.get_axon_ntff_profile_hook = lambda: hook
    mod.set_axon_ntff_profile_hook = lambda h: None
    sys.modules["antenv.axon_hooks"] = mod
    antenv.axon_hooks = mod


_last = {"exec_time_ns": None}


def kernel(q_alpha, k, h_norm, gamma_vec, causal_mask, decay_diff,
           _trace=False):
    trace = _trace or os.environ.get("BD_TRACE", "0") == "1"
    from concourse.bass_utils import run_bass_kernel_spmd

    nc = _build_program()
    in_maps = _host_prep(q_alpha, k, h_norm, gamma_vec, causal_mask)
    kwargs = {}
    if trace:
        _ensure_ntff_hook()
        import concourse.bass_utils as bu
        bu.upload_artifacts = lambda tmpdir: tmpdir  # no bucket in container
        kwargs = dict(trace=True, tmpdir=os.environ.get("BD_TRACE_DIR") or None)
    res = run_bass_kernel_spmd(nc, in_maps, list(range(B)), **kwargs)
    _last["exec_time_ns"] = res.exec_time_ns
    out = np.empty((B, W, D), np.float32)
    for b in range(B):
        out[b] = res.results[b]["otT"].T.astype(np.float32)
    return out


# revision 6
# speedup vs baseline: 1.2780x; 1.2780x over previous
"""BlockDecay (RetNet-style chunkwise linear attention with per-feature decay)
Trainium2 Bass kernel, batch-parallel over 8 NeuronCores.

Math (per batch): out[t] = sum_r q[t,r] * S_t[r,:],
  S_t[r,d] = sum_{s<=t} gamma_r^{t-s} k[s,r] h[s,d]
computed chunkwise with C=256 using the standard factorization
  A[i,j] = (q gamma^i) . (k gamma^-j),  intra = (A*mask) @ h,
  inter  = (q gamma^i) @ S,   S' = gamma^C S + K',
  K'[r,d] = sum_j gamma_r^{C-j} k[j,r] h[j,d]   (S carries a folded
  gamma^1 so inter needs no extra scale)

v14: all-bf16 PE stream (1 cyc/row, FWL weight loads; PSUM stays fp32).
C=256 superchunks halve the serial DVE state-update count (the per-op
cost is fixed-dominated).  Per superchunk the A block splits into
  A0 [j0, 256i] (left 128 cols tri-masked, right unmasked),
  A1 [j1, 128i] (right half only, tri-masked; the left half is
  structurally zero and skipped via an N=128 intra matmul).
Two superchunks share one [128,1024] PSUM tile (unit layout
A0|A0r|A1r|pad per superchunk) so ONE strided DVE tensor_mul masks all
four triangles against a x4-replicated tri, and ONE strided ACT copy
moves the two unmasked A0-right units.  PE work for output blocks is
emitted one group late (pend) so the PE queue never stalls behind the
mask.  Inputs ride the gpsimd(SWDGE)+sync(HWDGE) queues; outputs and
consts ride scalar's separate HWDGE queue (sharing an output queue
with inputs was measured to halve that queue's input bandwidth).

Host pre-scales/transposes all operands; device layout:
  qsT [R, W] bf16 = (q * gamma^(i%C)).T
  ksT [R, W] bf16 = (k * gamma^-(j%C)).T
  k2n [128, W] bf16  block-local [j, (blk, r)] = k*gamma^(C - j%C)
  hn  [128, W] bf16  block-local [j, (blk, d)]
  tri [128, 128] f32 causal mask transposed (tri[j,i] = i>=j)
  g256 [128, 1] f32 = gamma^C
Output otT [D, W] bf16 (transposed), host transposes + upcasts.
Measured rel err (absmax norm) ~5e-3 vs the 2e-2 gate.
"""
import os
import sys
import numpy as np

for _p in ("/root/.axon_site", "/root/.axon_site/_ro/trn_rl_repo",
           "/root/.axon_site/_ro/pypackages"):
    if _p not in sys.path and os.path.isdir(_p):
        sys.path.append(_p)

B, W, R, D = 8, 4096, 128, 128
C = 256
NSC = W // C          # 16 superchunks
NG = NSC // 2         # 8 groups of 2 superchunks

# output DMA pieces in groups (512 cols each): piece [lo,hi) groups,
# issued after group hi-1's OT copy
_OUT_PIECES = {1: (0, 2), 3: (2, 4), 5: (4, 6), 6: (6, 7), 7: (7, 8)}

_PROG = {}


def _patched_tc(nc):
    """TileContext with a cheap exit: per-sem single-wait drains on sync
    (this walrus accepts one sync-wait per instruction, and a blocking
    drain on an early-finishing engine stalls SWDGE descriptor handling),
    one barrier, then sem clears for idempotent re-execution.  The final
    join is walrus's own BSP model-end sync."""
    import concourse.tile as tile
    import concourse.tile_sem_assignment as tsa
    from concourse.tile import ScopedClock

    class PatchedTileContext(tile.TileContext):
        def _drain_and_barrier(self, tick_clock, wait_clock):
            gc = tick_clock.global_clock
            n = tsa.N_PROCS
            nc = self.nc
            for p in range(n):
                ticks = gc[p]
                if ticks <= 0:
                    continue
                d = nc.sync.drain()
                wait_clock.add_sem_waits(
                    d.ins,
                    ScopedClock({None: tsa.VectorClock(
                        [ticks if q == p else 0 for q in range(n)])}),
                )
            nc.all_engine_barrier()
            assert self.sems is not None
            popped = nc._tile_sem_poison_stack.pop()
            assert popped is self._sem_poison
            nc.clear_and_free_semaphores(list(self.sems.allocated().values()))

    return PatchedTileContext(nc)


def _split_multi_waits(nc, limit=1):
    """Hoist extra sync-waits onto injected same-engine NoOps (in-order
    engines make waiting earlier in the stream safe)."""
    import concourse.mybir as mybir
    n_new = 0
    for fn in nc.m.functions:
        for bb in fn.blocks:
            out = []
            changed = False
            for inst in bb.instructions:
                si = getattr(inst, "sync_info", None)
                waits = list(si.on_wait) if si is not None and si.on_wait else []
                if len(waits) > limit:
                    for w in waits[:-limit]:
                        nop = mybir.InstNoOp(
                            name=f"I-wsplit-{n_new}",
                            engine=inst.engine,
                            sync_info=mybir.SyncInfo(on_wait=[w], on_update=[]),
                        )
                        n_new += 1
                        out.append(nop)
                    si.on_wait = waits[-limit:]
                    changed = True
                out.append(inst)
            if changed:
                bb.instructions = out
    return n_new


def _build_program():
    key = "v14"
    if key in _PROG:
        return _PROG[key]
    import concourse.bass as bass
    import concourse.mybir as mybir

    F32 = mybir.dt.float32
    BF = mybir.dt.bfloat16
    n_warm = int(os.environ.get("BD_NWARM", "5"))

    nc = bass.Bass()
    qsT = nc.declare_dram_parameter("qsT", [128, W], BF, isOutput=False)
    ksT = nc.declare_dram_parameter("ksT", [128, W], BF, isOutput=False)
    k2n = nc.declare_dram_parameter("k2n", [128, W], BF, isOutput=False)
    hn = nc.declare_dram_parameter("hn", [128, W], BF, isOutput=False)
    tri = nc.declare_dram_parameter("tri", [128, 128], F32, isOutput=False)
    g256 = nc.declare_dram_parameter("g256", [128, 1], F32, isOutput=False)
    otT = nc.declare_dram_parameter("otT", [128, W], BF, isOutput=True)

    mm = nc.tensor.matmul

    def units(ap, start, step, n):
        # [128, U*128] -> strided pick of n 128-col units from `start`,
        # every `step` units -> [128, n, 128]
        r = ap.rearrange("p (u c) -> p u c", c=128)
        return r[:, start:start + step * (n - 1) + 1:step, :]

    with _patched_tc(nc) as tc:
        with tc.tile_pool(name="big", bufs=1) as big, \
             tc.tile_pool(name="small", bufs=1) as small, \
             tc.tile_pool(name="st", bufs=6) as stp, \
             tc.tile_pool(name="amp", bufs=3) as amp, \
             tc.tile_pool(name="ps_at", bufs=2, space="PSUM") as ps_at, \
             tc.tile_pool(name="ps_ot", bufs=2, space="PSUM") as ps_ot, \
             tc.tile_pool(name="ps_kp", bufs=2, space="PSUM") as ps_kp:

            qsT_sb = big.tile([128, W], BF, tag="qsT")
            ksT_sb = big.tile([128, W], BF, tag="ksT")
            k2n_sb = big.tile([128, W], BF, tag="k2n")
            hn_sb = big.tile([128, W], BF, tag="hn")
            otT_sb = big.tile([128, W], BF, tag="otT")
            tri_sb = small.tile([128, 128], F32, tag="tri")
            tri4_sb = small.tile([128, 512], BF, tag="tri4")
            g256_sb = small.tile([128, 1], F32, tag="g256")

            # PE warm-up: dummy matmuls fill the DMA-wait window and help
            # flip the HAM clock gate to 8/8 before the real stream.
            wz = small.tile([128, 512], BF, tag="wz")
            nc.vector.memset(wz[:], 0.0)
            for _ in range(n_warm):
                wp = ps_ot.tile([128, 512], F32, tag="ot")
                mm(wp[:], wz[:, :128], wz[:], start=True, stop=True)

            # inputs: KP path (k2n, hn) on gpsimd/SWDGE (starts first),
            # A path (ksT, qsT) on sync/HWDGE; consts + outputs on
            # scalar's separate HWDGE queue.  Head pieces of 256 cols get
            # compute started ~1.5us earlier; then 1024-col pieces.
            nc.scalar.dma_start(tri_sb[:], tri[:])
            nc.scalar.dma_start(g256_sb[:], g256[:])
            bounds = [0, 256, 1024, 2048, 3072, 4096]
            for p in range(len(bounds) - 1):
                s = slice(bounds[p], bounds[p + 1])
                nc.gpsimd.dma_start(k2n_sb[:, s], k2n[:, s])
                nc.gpsimd.dma_start(hn_sb[:, s], hn[:, s])
                nc.sync.dma_start(ksT_sb[:, s], ksT[:, s])
                nc.sync.dma_start(qsT_sb[:, s], qsT[:, s])

            # tri4 = bf16 causal mask replicated x4 (doubling build)
            nc.scalar.copy(tri4_sb[:, 0:128], tri_sb[:])
            nc.scalar.copy(tri4_sb[:, 128:256], tri4_sb[:, 0:128])
            nc.scalar.copy(tri4_sb[:, 256:512], tri4_sb[:, 0:256])

            S_prev = stp.tile([128, 128], BF, tag="S")
            nc.vector.memset(S_prev[:], 0.0)
            S_at = {0: S_prev}

            pend = None
            for g in range(NG):
                # --- state path: K' matmuls + serial DVE update ---
                for s in (2 * g, 2 * g + 1):
                    j0 = slice(s * C, s * C + 128)
                    j1 = slice(s * C + 128, s * C + 256)
                    kp = ps_kp.tile([128, 128], F32, tag="kp")
                    mm(kp[:], k2n_sb[:, j0], hn_sb[:, j0], start=True,
                       stop=False)
                    mm(kp[:], k2n_sb[:, j1], hn_sb[:, j1], start=False,
                       stop=True)
                    S_new = stp.tile([128, 128], BF, tag="S")
                    nc.vector.scalar_tensor_tensor(
                        out=S_new[:], in0=S_at[s][:], scalar=g256_sb[:, 0:1],
                        in1=kp[:], op0=mybir.AluOpType.mult,
                        op1=mybir.AluOpType.add)
                    S_at[s + 1] = S_new

                # --- A blocks for both superchunks into one PSUM tile ---
                # unit layout per superchunk: [A0(2) | A1r(1) | pad(1)]
                atb = ps_at.tile([128, 1024], F32, tag="at")
                for h, s in enumerate((2 * g, 2 * g + 1)):
                    ci = slice(s * C, (s + 1) * C)
                    i1 = slice(s * C + 128, (s + 1) * C)
                    j0 = slice(s * C, s * C + 128)
                    j1 = slice(s * C + 128, s * C + 256)
                    u = h * 512
                    mm(atb[:, u:u + 256], ksT_sb[:, j0], qsT_sb[:, ci],
                       start=True, stop=True)
                    mm(atb[:, u + 256:u + 384], ksT_sb[:, j1],
                       qsT_sb[:, i1], start=True, stop=True)
                am = amp.tile([128, 1024], BF, tag="am")
                # one op masks all four triangles (A0-left + A1r, x2)
                nc.vector.tensor_mul(units(am[:], 0, 2, 4),
                                     units(atb[:], 0, 2, 4),
                                     tri4_sb[:].rearrange(
                                         "p (u c) -> p u c", c=128))
                # unmasked A0-right units move via ACT
                nc.scalar.copy(units(am[:], 1, 4, 2), units(atb[:], 1, 4, 2))

                # --- output blocks, one group late (keeps PE busy while
                # the mask runs) ---
                if pend is not None:
                    _emit_out(nc, mm, pend, hn_sb, qsT_sb, otT_sb, otT,
                              ps_ot, S_at)
                pend = (g, am)
            _emit_out(nc, mm, pend, hn_sb, qsT_sb, otT_sb, otT, ps_ot, S_at)

    _split_multi_waits(nc)
    _PROG[key] = nc
    return nc


def _emit_out(nc, mm, pend, hn_sb, qsT_sb, otT_sb, otT, ps_ot, S_at):
    import concourse.mybir as mybir
    g, am = pend
    ot = ps_ot.tile([128, 512], mybir.dt.float32, tag="ot")
    for h, s in enumerate((2 * g, 2 * g + 1)):
        ci = slice(s * C, (s + 1) * C)
        j0 = slice(s * C, s * C + 128)
        j1 = slice(s * C + 128, s * C + 256)
        u = h * 512
        o = h * 256
        mm(ot[:, o:o + 256], hn_sb[:, j0], am[:, u:u + 256],
           start=True, stop=False)
        mm(ot[:, o + 128:o + 256], hn_sb[:, j1], am[:, u + 256:u + 384],
           start=False, stop=False)
        mm(ot[:, o:o + 256], S_at[s][:], qsT_sb[:, ci],
           start=False, stop=True)
    oj = slice(g * 512, (g + 1) * 512)
    nc.scalar.copy(otT_sb[:, oj], ot[:])
    if g in _OUT_PIECES:
        lo, hi = _OUT_PIECES[g]
        s = slice(lo * 512, hi * 512)
        nc.scalar.dma_start(otT[:, s], otT_sb[:, s])


def _host_prep(q_alpha, k, h_norm, gamma_vec, causal_mask):
    import ml_dtypes
    bf = ml_dtypes.bfloat16
    gamma = np.clip(np.asarray(gamma_vec, np.float64), 1e-8, None)
    log_g = np.log(gamma)
    i_loc = (np.arange(W) % C).astype(np.float64)
    Sq = np.exp(np.outer(i_loc, log_g))          # [W, R] gamma^(i%C)
    Skneg = np.exp(np.outer(-i_loc, log_g))      # gamma^-(j%C)
    Sk2 = np.exp(np.outer(C - i_loc, log_g))     # gamma^(C - j%C)
    g256 = np.exp(C * log_g).astype(np.float32).reshape(128, 1)

    tri = np.ascontiguousarray(np.asarray(causal_mask, np.float32).T)

    def blockify(x):  # [W, 128] -> [128, (blk, 128)]
        nb = W // 128
        return np.ascontiguousarray(
            x.reshape(nb, 128, 128).transpose(1, 0, 2).reshape(128, W))

    in_maps = []
    for b in range(B):
        q64 = np.asarray(q_alpha[b], np.float64)
        k64 = np.asarray(k[b], np.float64)
        in_maps.append({
            "qsT": np.ascontiguousarray((q64 * Sq).T.astype(bf)),
            "ksT": np.ascontiguousarray((k64 * Skneg).T.astype(bf)),
            "k2n": blockify((k64 * Sk2).astype(bf)),
            "hn": blockify(np.asarray(h_norm[b], bf)),
            "tri": tri,
            "g256": g256,
        })
    return in_maps


def _ensure_ntff_hook():
    try:
        from antenv import axon_hooks  # noqa: F401
        return
    except ImportError:
        pass
    import types
    import antenv
    try:
        import trn_agent_boot.trn_boot as tb
        hook = tb._ntff_profile_via_ctypes("/opt/axon/libaxon_pjrt.so")
    except Exception:
        hook = None
    mod = types.ModuleType("antenv.axon_hooks")
    mod.get_axon_ntff_profile_hook = lambda: hook
    mod.set_axon_ntff_profile_hook = lambda h: None
    sys.modules["antenv.axon_hooks"] = mod
    antenv.axon_hooks = mod


_last = {"exec_time_ns": None}


def kernel(q_alpha, k, h_norm, gamma_vec, causal_mask, decay_diff,
           _trace=False):
    trace = _trace or os.environ.get("BD_TRACE", "0") == "1"
    from concourse.bass_utils import run_bass_kernel_spmd

    nc = _build_program()
    in_maps = _host_prep(q_alpha, k, h_norm, gamma_vec, causal_mask)
    kwargs = {}
    if trace:
        _ensure_ntff_hook()
        import concourse.bass_utils as bu
        bu.upload_artifacts = lambda tmpdir: tmpdir  # no bucket in container
        kwargs = dict(trace=True, tmpdir=os.environ.get("BD_TRACE_DIR") or None)
    res = run_bass_kernel_spmd(nc, in_maps, list(range(B)), **kwargs)
    _last["exec_time_ns"] = res.exec_time_ns
    out = np.empty((B, W, D), np.float32)
    for b in range(B):
        out[b] = res.results[b]["otT"].T.astype(np.float32)
    return out
